# revision 29
# baseline (speedup 1.0000x reference)
"""Trainium2 Bass kernel for shared-QK attention (Q=K) with probs output.

Reference computation (B=2, S=2048, D=1024, H=16, U=64):
    qk    = inputs @ W_qk + b_qk            -> [B,H,S,U]
    v     = inputs @ W_v  + b_v             -> [B,H,S,U]
    s     = qk @ qk^T / sqrt(U)             -> [B,H,S,S]   (symmetric)
    s    += eye-exclusion mask (-1e4 on diagonal; mask input is all-ones)
    probs = softmax(s, -1)                  -> [B,H,S,S]
    ctx   = probs @ v                       -> [B,S,H*U]
    returns (ctx, probs)

Sharding: 2 heads per core x 8 cores (both batches local), no collectives —
each core emits its probs/context shard as an ExternalOutput and the host
assembles the full tensors.

Per-core kernel design notes:
  - inputs are PE-transposed (f32) into inputsT (bf16), feeding both
    projections with D on partitions.
  - exp(scores) is computed unnormalized (no max subtraction needed: |s|<~4).
    E := exp(s/8) is symmetric, so stored E row-tiles [q,k] double as E^T
    k-chunk tiles [k,q] for the context matmul — no transpose of probs.
  - ACT's fused accum_out on the exp gives the row-sums for free; probs are
    E * (1/rowsum) via per-partition tensor_scalar, context is normalized
    after a PE transpose back to [s,u] layout (again per-partition scalars).
  - diagonal exclusion: add -8e4*I to the score PSUM tile pre-exp, so
    exp((s-8e4)/8) underflows to exactly 0.0 like the reference.
"""

import numpy as np

NUM_HEADS = 16
UNITS = 64
B, S, D = 2, 2048, 1024
N_CORES = 8
HL = NUM_HEADS // N_CORES  # heads per core (2)
BS = B * S  # 4096

# engine for the probs normalization pass per q-block parity:
#   "v" = vector (DVE, tensor_tensor w/ stride-0 broadcast), "s" = scalar (ACT,
#   fused scale on a Copy activation). gpsimd/tensor_scalar-with-AP measured
#   ~11 cyc/elem — do not use.
P_SCALE_ENGINES = ("v",)
# probs DRAM/store dtype: bf16 halves the dominant DMA stream; host converts
# back to f32. Set to "f32" to store probs in full precision.
PROBS_DTYPE = "bf16"

_COMPILED = {}


def _build():
    import concourse.bass as bass  # noqa: F401
    import concourse.mybir as mybir
    from concourse import bacc
    from concourse.tile import TileContext
    from concourse.masks import make_identity
    from contextlib import ExitStack

    dt = mybir.dt
    f32 = dt.float32
    bf16 = dt.bfloat16
    AF = mybir.ActivationFunctionType
    ALU = mybir.AluOpType

    U = UNITS
    DC = D // 128          # 8 contraction chunks
    NSB = BS // 512        # 8 s-blocks of 512
    QB = S // 128          # 16 q-blocks per (b,h)
    KC = S // 512          # 4 free-dim chunks of 512

    nc = bacc.Bacc("TRN2", target_bir_lowering=False, debug=False)
    inputs_ext = nc.declare_dram_parameter("inputs", [BS, D], f32, isOutput=False)
    wqk_ext = nc.declare_dram_parameter("w_qk", [D, HL * U], f32, isOutput=False)
    bqk_ext = nc.declare_dram_parameter("b_qk", [HL * U, 1], f32, isOutput=False)
    wv_ext = nc.declare_dram_parameter("w_v", [D, HL * U], f32, isOutput=False)
    bv_ext = nc.declare_dram_parameter("b_v", [HL * U, 1], f32, isOutput=False)
    probs_dt = bf16 if PROBS_DTYPE == "bf16" else f32
    probs_ext = nc.declare_dram_parameter(
        "probs", [B, HL, S, S], probs_dt, isOutput=True
    )
    # context is exported transposed+unnormalized ([u, q] straight from PSUM)
    # together with the per-row reciprocals; the host applies the (tiny)
    # transpose + row scaling during unsharding.
    ctxt_ext = nc.declare_dram_parameter(
        "ctxT", [B, HL, U, S], f32, isOutput=True
    )
    recip_ext = nc.declare_dram_parameter(
        "recips", [B, HL, 128, S // 128], f32, isOutput=True
    )

    with TileContext(nc) as tc, ExitStack() as top:
        consts = top.enter_context(tc.tile_pool(name="consts", bufs=1))
        pers = top.enter_context(tc.tile_pool(name="pers", bufs=1))

        ident_f32 = consts.tile([128, 128], f32, tag="idf32")
        make_identity(nc, ident_f32[:])
        ident_bf16 = consts.tile([128, 128], bf16, tag="idbf16")
        make_identity(nc, ident_bf16[:])
        # -8e4 * I as bf16, added into the scores diagonal via a PE matmul
        # (ident.T @ eyeneg) so the exclusion never leaves the TensorEngine
        eyeneg = consts.tile([128, 128], bf16, tag="eyeneg")
        make_identity(nc, eyeneg[:])
        nc.scalar.mul(eyeneg[:], eyeneg[:], -80000.0)

        bqk_col = consts.tile([128, 1], f32, tag="bqk")
        nc.sync.dma_start(out=bqk_col[:], in_=bqk_ext[:, :])
        bv_col = consts.tile([128, 1], f32, tag="bv")
        nc.sync.dma_start(out=bv_col[:], in_=bv_ext[:, :])

        # weights: [D, 128] -> sbuf [128(d within chunk), DC*128(u)] as bf16
        wqk_sb = consts.tile([128, DC * 128], bf16, tag="wqk")
        wv_sb = consts.tile([128, DC * 128], bf16, tag="wv")
        for w_ext, w_sb, tag in ((wqk_ext, wqk_sb, "wqkf"), (wv_ext, wv_sb, "wvf")):
            w_f32 = consts.tile([128, DC * 128], f32, tag=tag)
            for c in range(DC):
                nc.sync.dma_start(
                    out=w_f32[:, c * 128 : (c + 1) * 128],
                    in_=w_ext[c * 128 : (c + 1) * 128, :],
                )
            nc.vector.tensor_copy(w_sb[:], w_f32[:])

        # persistent intermediates
        qkT = [pers.tile([64, BS], bf16, tag=f"qkT{h}", name=f"qkT{h}") for h in range(HL)]
        vloc = [pers.tile([128, (BS // 128) * U], bf16, tag=f"v{h}", name=f"vloc{h}") for h in range(HL)]

        # ---------------- Phase 1: transpose + projections ----------------
        with ExitStack() as p1:
            in_pool = p1.enter_context(tc.tile_pool(name="inp", bufs=8))
            it_pool = p1.enter_context(tc.tile_pool(name="inT", bufs=1))
            vt_pool = p1.enter_context(tc.tile_pool(name="vT", bufs=2))
            ptr_pool = p1.enter_context(tc.tile_pool(name="ptr", bufs=2, space="PSUM"))
            pqk_pool = p1.enter_context(tc.tile_pool(name="pqk", bufs=2, space="PSUM"))
            pv_pool = p1.enter_context(tc.tile_pool(name="pv", bufs=2, space="PSUM"))
            pvt_pool = p1.enter_context(tc.tile_pool(name="pvt", bufs=2, space="PSUM"))

            inputsT = [
                it_pool.tile([128, BS], bf16, tag=f"inT{dc}", name=f"inT{dc}")
                for dc in range(DC)
            ]

            for sb in range(NSB):
                s0 = sb * 512
                in_tiles = []
                for t in range(4):
                    it = in_pool.tile([128, D], f32, tag="in")
                    nc.sync.dma_start(
                        out=it[:], in_=inputs_ext[s0 + t * 128 : s0 + (t + 1) * 128, :]
                    )
                    # cast to bf16 up front: PE transposes run 2x faster on
                    # bf16 and the projections consume bf16 anyway
                    ib = in_pool.tile([128, D], bf16, tag="inb")
                    if t % 2 == 0:
                        nc.vector.tensor_copy(ib[:], it[:])
                    else:
                        nc.scalar.copy(ib[:], it[:])
                    in_tiles.append(ib)

                # transpose this s-block: inputsT[dc][:, s0:s0+512]
                for dc in range(DC):
                    ptr = ptr_pool.tile([128, 512], bf16, tag="ptr")
                    for t in range(4):
                        nc.tensor.transpose(
                            ptr[:, t * 128 : (t + 1) * 128],
                            in_tiles[t][:, dc * 128 : (dc + 1) * 128],
                            ident_bf16[:],
                        )
                    dst = inputsT[dc][:, s0 : s0 + 512]
                    if dc % 2 == 0:
                        nc.vector.tensor_copy(dst, ptr[:])
                    else:
                        nc.scalar.copy(dst, ptr[:])

                # qk projection -> qkT (u on partitions), + bias
                pqk = pqk_pool.tile([128, 512], f32, tag="pqk")
                for dc in range(DC):
                    nc.tensor.matmul(
                        pqk[:],
                        wqk_sb[:, dc * 128 : (dc + 1) * 128],
                        inputsT[dc][:, s0 : s0 + 512],
                        start=(dc == 0),
                        stop=(dc == DC - 1),
                    )
                for h in range(HL):
                    nc.vector.tensor_scalar_add(
                        qkT[h][:, s0 : s0 + 512],
                        pqk[h * 64 : (h + 1) * 64, :],
                        bqk_col[h * 64 : (h + 1) * 64, :],
                    )

                # v projection -> vT, + bias, then PE-transpose to [s,u] chunks
                pv = pv_pool.tile([128, 512], f32, tag="pv")
                for dc in range(DC):
                    nc.tensor.matmul(
                        pv[:],
                        wv_sb[:, dc * 128 : (dc + 1) * 128],
                        inputsT[dc][:, s0 : s0 + 512],
                        start=(dc == 0),
                        stop=(dc == DC - 1),
                    )
                vt_sb = vt_pool.tile([128, 512], bf16, tag="vt")
                for h in range(HL):
                    nc.scalar.activation(
                        vt_sb[h * 64 : (h + 1) * 64, :],
                        pv[h * 64 : (h + 1) * 64, :],
                        AF.Identity,
                        bias=bv_col[h * 64 : (h + 1) * 64, :],
                    )
                for h in range(HL):
                    for t in range(4):
                        ci = sb * 4 + t  # global 128-row chunk index in [0,32)
                        pvt = pvt_pool.tile([128, 64], bf16, tag="pvt")
                        nc.tensor.transpose(
                            pvt[:],
                            vt_sb[h * 64 : (h + 1) * 64, t * 128 : (t + 1) * 128],
                            ident_bf16[h * 64 : (h + 1) * 64, h * 64 : (h + 1) * 64],
                        )
                        nc.vector.tensor_copy(
                            vloc[h][:, ci * 64 : (ci + 1) * 64], pvt[:]
                        )

        # ---------------- Phase 2: attention per (b, h) ----------------
        with ExitStack() as p2:
            # PSUM budget (8 banks): scores split lo/hi so score matmuls of
            # q-block i+1 overlap the exp of q-block i: lo [128,1024] x2 bufs
            # (4 banks) + hi [128,1024] x1 (2 banks) + ctx packed into 2 banks
            # (4 chunks of [64,512] on 2 partition-halves x 2 bank-halves).
            scl_pool = p2.enter_context(tc.tile_pool(name="scl", bufs=2, space="PSUM"))
            sch_pool = p2.enter_context(tc.tile_pool(name="sch", bufs=1, space="PSUM"))
            cx_pool = p2.enter_context(tc.tile_pool(name="cx", bufs=1, space="PSUM"))
            e_pool = p2.enter_context(tc.tile_pool(name="e", bufs=3))
            p_pool = p2.enter_context(tc.tile_pool(name="p", bufs=4))
            acc_pool = p2.enter_context(tc.tile_pool(name="acc", bufs=4))
            r_pool = p2.enter_context(tc.tile_pool(name="recip", bufs=2))
            ct_pool = p2.enter_context(tc.tile_pool(name="ctxT", bufs=2))
            co_pool = p2.enter_context(tc.tile_pool(name="cout", bufs=4))

            for b in range(B):
                for h in range(HL):
                    base = b * S
                    recips = r_pool.tile([128, QB], f32, tag="recips")
                    cx = cx_pool.tile([128, S // 2], f32, tag="cx")

                    def cx_sl(c):
                        # ctx chunk c of [64,512] packed: partition-half c%2,
                        # bank-half c//2
                        p0 = (c % 2) * 64
                        f0 = (c // 2) * 512
                        return cx[p0 : p0 + 64, f0 : f0 + 512]

                    def ctx_mms(qb, e_t, b=b, h=h):
                        # context accumulation: E row-tile doubles as E^T k-chunk
                        for c in range(KC):
                            nc.tensor.matmul(
                                cx_sl(c),
                                vloc[h][:, (b * QB + qb) * 64 : (b * QB + qb + 1) * 64],
                                e_t[:, c * 512 : (c + 1) * 512],
                                start=(qb == 0),
                                stop=(qb == QB - 1),
                            )

                    def score_mm(dst, c, qb, lhsT_s, h=h):
                        diag = c == qb // 4
                        nc.tensor.matmul(
                            dst[:, (c % 2) * 512 : (c % 2) * 512 + 512],
                            lhsT_s,
                            qkT[h][:, base + c * 512 : base + (c + 1) * 512],
                            start=True,
                            stop=not diag,
                        )
                        if diag:
                            # diagonal exclusion (pre-exp: row-sums exclude
                            # it): accumulate ident.T @ (-8e4*I) = -8e4*I
                            o = qb * 128 - (c // 2) * 1024
                            nc.tensor.matmul(
                                dst[:, o : o + 128],
                                ident_bf16[:],
                                eyeneg[:],
                                start=False,
                                stop=True,
                            )

                    prev = None  # (qb, e_t) pending context matmuls
                    for qb in range(QB):
                        sc_lo = scl_pool.tile([128, S // 2], f32, tag="scl")
                        sc_hi = sch_pool.tile([128, S // 2], f32, tag="sch")
                        lhsT_s = qkT[h][:, base + qb * 128 : base + (qb + 1) * 128]
                        for c in (0, 1):
                            score_mm(sc_lo, c, qb, lhsT_s)
                        for c in (2, 3):
                            score_mm(sc_hi, c, qb, lhsT_s)
                        e_t = e_pool.tile([128, S], bf16, tag="e")
                        acc_a = acc_pool.tile([128, 1], f32, tag="acca")
                        acc_b = acc_pool.tile([128, 1], f32, tag="accb")
                        nc.scalar.activation(
                            e_t[:, 0 : S // 2], sc_lo[:], AF.Exp, bias=0.0,
                            scale=0.125, accum_out=acc_a[:],
                        )
                        nc.scalar.activation(
                            e_t[:, S // 2 : S], sc_hi[:], AF.Exp, bias=0.0,
                            scale=0.125, accum_out=acc_b[:],
                        )
                        # previous q-block's context matmuls: ready PE work
                        # while this exp (and the probs scale) runs
                        if prev is not None:
                            ctx_mms(*prev)
                        prev = (qb, e_t)
                        # HAM filler: a discarded matmul into the just-read
                        # scores slot keeps the PE activity monitor from
                        # dropping the clock to 1.2GHz during the exp window
                        nc.tensor.matmul(
                            sc_lo[:, 0:512],
                            lhsT_s,
                            qkT[h][:, base : base + 512],
                            start=True,
                            stop=True,
                        )
                        nc.vector.tensor_add(acc_a[:], acc_a[:], acc_b[:])
                        nc.vector.reciprocal(recips[:, qb : qb + 1], acc_a[:])
                        # probs = E * (1/rowsum), streamed to DRAM
                        p_t = p_pool.tile([128, S], probs_dt, tag="p")
                        eng = P_SCALE_ENGINES[qb % len(P_SCALE_ENGINES)]
                        if eng == "s":
                            nc.scalar.activation(
                                p_t[:], e_t[:], AF.Copy,
                                scale=recips[:, qb : qb + 1],
                            )
                        else:
                            # bf16-in/bf16-out tensor_scalar hits the packed
                            # DVE mode (~750ns/tile vs 2.3us for broadcast TT)
                            nc.vector.tensor_scalar_mul(
                                p_t[:], e_t[:], recips[:, qb : qb + 1]
                            )
                        nc.sync.dma_start(
                            out=probs_ext[b, h, qb * 128 : (qb + 1) * 128, :],
                            in_=p_t[:],
                        )
                    ctx_mms(*prev)
                    # finalize context: copy accumulated [u, q] out as-is; the
                    # host transposes + row-normalizes this 2MB side output
                    ctxT_sb = ct_pool.tile([64, S], f32, tag="ctxT")
                    for c in range(KC):
                        nc.scalar.copy(ctxT_sb[:, c * 512 : (c + 1) * 512], cx_sl(c))
                    nc.sync.dma_start(out=ctxt_ext[b, h, :, :], in_=ctxT_sb[:])
                    nc.sync.dma_start(out=recip_ext[b, h, :, :], in_=recips[:])

    nc.compile()
    return nc


def _get_compiled():
    if "nc" not in _COMPILED:
        _COMPILED["nc"] = _build()
    return _COMPILED["nc"]


def _numpy_fallback(inputs, mask, W_qk, b_qk, W_v, b_v):
    H, U = NUM_HEADS, UNITS
    x = inputs.astype(np.float64)
    qk = (x @ W_qk + b_qk).reshape(B, S, H, U).transpose(0, 2, 1, 3)
    v = (x @ W_v + b_v).reshape(B, S, H, U).transpose(0, 2, 1, 3)
    sc = np.einsum("bhqd,bhkd->bhqk", qk, qk) / np.sqrt(np.float64(U))
    m = np.ones((B, S, 1)) * mask.astype(np.float64)[:, None, :]
    m = np.clip(m - np.eye(S), 0.0, 1.0)
    sc = sc + (1.0 - m[:, None, :, :]) * -10000.0
    sc -= sc.max(-1, keepdims=True)
    e = np.exp(sc)
    probs = e / e.sum(-1, keepdims=True)
    ctx = np.einsum("bhqk,bhkd->bhqd", probs, v)
    ctx = ctx.transpose(0, 2, 1, 3).reshape(B, S, H * U)
    return ctx.astype(np.float32), probs.astype(np.float32)


def _make_in_maps(inputs, W_qk, b_qk, W_v, b_v):
    in_maps = []
    for c in range(N_CORES):
        cols = slice(c * HL * UNITS, (c + 1) * HL * UNITS)
        in_maps.append(
            {
                "inputs": np.ascontiguousarray(
                    inputs.reshape(BS, D).astype(np.float32)
                ),
                "w_qk": np.ascontiguousarray(W_qk[:, cols].astype(np.float32)),
                "b_qk": np.ascontiguousarray(
                    b_qk[cols].astype(np.float32).reshape(HL * UNITS, 1)
                ),
                "w_v": np.ascontiguousarray(W_v[:, cols].astype(np.float32)),
                "b_v": np.ascontiguousarray(
                    b_v[cols].astype(np.float32).reshape(HL * UNITS, 1)
                ),
            }
        )
    return in_maps


def _assemble(results):
    probs = np.empty((B, NUM_HEADS, S, S), np.float32)
    ctx = np.empty((B, S, NUM_HEADS * UNITS), np.float32)
    for c in range(N_CORES):
        pr = np.asarray(results[c]["probs"]).astype(np.float32).reshape(B, HL, S, S)
        probs[:, c * HL : (c + 1) * HL] = pr
        ctxT = np.asarray(results[c]["ctxT"]).reshape(B, HL, UNITS, S)
        recips = np.asarray(results[c]["recips"]).reshape(B, HL, 128, S // 128)
        for b in range(B):
            for h in range(HL):
                r = recips[b, h].T.reshape(S)  # r[q], q = qb*128 + p
                col = (c * HL + h) * UNITS
                ctx[b, :, col : col + UNITS] = (ctxT[b, h] * r[None, :]).T
    return ctx, probs


def kernel(inputs, mask, W_qk, b_qk, W_v, b_v):
    inputs = np.asarray(inputs, np.float32)
    mask = np.asarray(mask)
    W_qk = np.asarray(W_qk, np.float32)
    b_qk = np.asarray(b_qk, np.float32)
    W_v = np.asarray(W_v, np.float32)
    b_v = np.asarray(b_v, np.float32)
    if not bool(np.all(mask)):
        # the device kernel hardcodes the all-ones mask fast path
        return _numpy_fallback(inputs, mask, W_qk, b_qk, W_v, b_v)

    from concourse.bass_utils import run_bass_kernel_spmd

    nc = _get_compiled()
    in_maps = _make_in_maps(inputs, W_qk, b_qk, W_v, b_v)
    res = run_bass_kernel_spmd(nc, in_maps, list(range(N_CORES)))
    ctx, probs = _assemble(res.results)
    return ctx, probs


# revision 31
# speedup vs baseline: 1.1081x; 1.1081x over previous
"""Trainium2 Bass kernel for shared-QK attention (Q=K) with probs output.

Reference computation (B=2, S=2048, D=1024, H=16, U=64):
    qk    = inputs @ W_qk + b_qk            -> [B,H,S,U]
    v     = inputs @ W_v  + b_v             -> [B,H,S,U]
    s     = qk @ qk^T / sqrt(U)             -> [B,H,S,S]   (symmetric)
    s    += eye-exclusion mask (-1e4 on diagonal; mask input is all-ones)
    probs = softmax(s, -1)                  -> [B,H,S,S]
    ctx   = probs @ v                       -> [B,S,H*U]
    returns (ctx, probs)

Sharding: 2 heads per core x 8 cores (both batches local), no collectives —
each core emits its probs/context shard as an ExternalOutput and the host
assembles the full tensors.

Per-core kernel design notes:
  - inputs are PE-transposed (f32) into inputsT (bf16), feeding both
    projections with D on partitions.
  - exp(scores) is computed unnormalized (no max subtraction needed: |s|<~4).
    E := exp(s/8) is symmetric, so stored E row-tiles [q,k] double as E^T
    k-chunk tiles [k,q] for the context matmul — no transpose of probs.
  - ACT's fused accum_out on the exp gives the row-sums for free; probs are
    E * (1/rowsum) via per-partition tensor_scalar, context is normalized
    after a PE transpose back to [s,u] layout (again per-partition scalars).
  - diagonal exclusion: add -8e4*I to the score PSUM tile pre-exp, so
    exp((s-8e4)/8) underflows to exactly 0.0 like the reference.
"""

import numpy as np

NUM_HEADS = 16
UNITS = 64
B, S, D = 2, 2048, 1024
N_CORES = 8
HL = NUM_HEADS // N_CORES  # heads per core (2)
BS = B * S  # 4096

# engine for the probs normalization pass per q-block parity:
#   "v" = vector (DVE, tensor_tensor w/ stride-0 broadcast), "s" = scalar (ACT,
#   fused scale on a Copy activation). gpsimd/tensor_scalar-with-AP measured
#   ~11 cyc/elem — do not use.
P_SCALE_ENGINES = ("v",)
# probs DRAM/store dtype: bf16 halves the dominant DMA stream; host converts
# back to f32. Set to "f32" to store probs in full precision.
PROBS_DTYPE = "bf16"

_COMPILED = {}


def _build():
    import concourse.bass as bass  # noqa: F401
    import concourse.mybir as mybir
    from concourse import bacc
    from concourse.tile import TileContext
    from concourse.masks import make_identity
    from contextlib import ExitStack

    dt = mybir.dt
    f32 = dt.float32
    bf16 = dt.bfloat16
    AF = mybir.ActivationFunctionType
    ALU = mybir.AluOpType
    ALU_X = mybir.AxisListType.X

    U = UNITS
    DC = D // 128          # 8 contraction chunks
    NSB = BS // 512        # 8 s-blocks of 512
    QB = S // 128          # 16 q-blocks per (b,h)
    KC = S // 512          # 4 free-dim chunks of 512

    nc = bacc.Bacc("TRN2", target_bir_lowering=False, debug=False)
    inputs_ext = nc.declare_dram_parameter("inputs", [BS, D], f32, isOutput=False)
    wqk_ext = nc.declare_dram_parameter("w_qk", [D, HL * U], f32, isOutput=False)
    bqk_ext = nc.declare_dram_parameter("b_qk", [HL * U, 1], f32, isOutput=False)
    wv_ext = nc.declare_dram_parameter("w_v", [D, HL * U], f32, isOutput=False)
    bv_ext = nc.declare_dram_parameter("b_v", [HL * U, 1], f32, isOutput=False)
    probs_dt = bf16 if PROBS_DTYPE == "bf16" else f32
    probs_ext = nc.declare_dram_parameter(
        "probs", [B, HL, S, S], probs_dt, isOutput=True
    )
    # context is exported transposed+unnormalized ([u, q] straight from PSUM)
    # together with the per-row reciprocals; the host applies the (tiny)
    # transpose + row scaling during unsharding.
    ctxt_ext = nc.declare_dram_parameter(
        "ctxT", [B, HL, U, S], f32, isOutput=True
    )
    recip_ext = nc.declare_dram_parameter(
        "recips", [B, HL, 128, S // 128], f32, isOutput=True
    )

    with TileContext(nc) as tc, ExitStack() as top:
        consts = top.enter_context(tc.tile_pool(name="consts", bufs=1))
        pers = top.enter_context(tc.tile_pool(name="pers", bufs=1))

        ident_f32 = consts.tile([128, 128], f32, tag="idf32")
        make_identity(nc, ident_f32[:])
        ident_bf16 = consts.tile([128, 128], bf16, tag="idbf16")
        make_identity(nc, ident_bf16[:])
        # -8e4 * I as bf16, added into the scores diagonal via a PE matmul
        # (ident.T @ eyeneg) so the exclusion never leaves the TensorEngine
        eyeneg = consts.tile([128, 128], bf16, tag="eyeneg")
        make_identity(nc, eyeneg[:])
        nc.scalar.mul(eyeneg[:], eyeneg[:], -80000.0)

        bqk_col = consts.tile([128, 1], f32, tag="bqk")
        nc.sync.dma_start(out=bqk_col[:], in_=bqk_ext[:, :])
        bv_col = consts.tile([128, 1], f32, tag="bv")
        nc.sync.dma_start(out=bv_col[:], in_=bv_ext[:, :])

        # weights: [D, 128] -> sbuf [128(d within chunk), DC*128(u)] as bf16
        wqk_sb = consts.tile([128, DC * 128], bf16, tag="wqk")
        wv_sb = consts.tile([128, DC * 128], bf16, tag="wv")
        for w_ext, w_sb, tag in ((wqk_ext, wqk_sb, "wqkf"), (wv_ext, wv_sb, "wvf")):
            w_f32 = consts.tile([128, DC * 128], f32, tag=tag)
            for c in range(DC):
                nc.sync.dma_start(
                    out=w_f32[:, c * 128 : (c + 1) * 128],
                    in_=w_ext[c * 128 : (c + 1) * 128, :],
                )
            nc.vector.tensor_copy(w_sb[:], w_f32[:])

        # persistent intermediates
        qkT = [pers.tile([64, BS], bf16, tag=f"qkT{h}", name=f"qkT{h}") for h in range(HL)]
        vloc = [pers.tile([128, (BS // 128) * U], bf16, tag=f"v{h}", name=f"vloc{h}") for h in range(HL)]

        # ---------------- Phase 1: transpose + projections ----------------
        with ExitStack() as p1:
            in_pool = p1.enter_context(tc.tile_pool(name="inp", bufs=8))
            it_pool = p1.enter_context(tc.tile_pool(name="inT", bufs=1))
            vt_pool = p1.enter_context(tc.tile_pool(name="vT", bufs=2))
            ptr_pool = p1.enter_context(tc.tile_pool(name="ptr", bufs=2, space="PSUM"))
            pqk_pool = p1.enter_context(tc.tile_pool(name="pqk", bufs=2, space="PSUM"))
            pv_pool = p1.enter_context(tc.tile_pool(name="pv", bufs=2, space="PSUM"))
            pvt_pool = p1.enter_context(tc.tile_pool(name="pvt", bufs=2, space="PSUM"))

            inputsT = [
                it_pool.tile([128, BS], bf16, tag=f"inT{dc}", name=f"inT{dc}")
                for dc in range(DC)
            ]

            for sb in range(NSB):
                s0 = sb * 512
                in_tiles = []
                for t in range(4):
                    it = in_pool.tile([128, D], f32, tag="in")
                    nc.sync.dma_start(
                        out=it[:], in_=inputs_ext[s0 + t * 128 : s0 + (t + 1) * 128, :]
                    )
                    # cast to bf16 up front: PE transposes run 2x faster on
                    # bf16 and the projections consume bf16 anyway
                    ib = in_pool.tile([128, D], bf16, tag="inb")
                    if t % 2 == 0:
                        nc.vector.tensor_copy(ib[:], it[:])
                    else:
                        nc.scalar.copy(ib[:], it[:])
                    in_tiles.append(ib)

                # transpose this s-block: inputsT[dc][:, s0:s0+512]
                for dc in range(DC):
                    ptr = ptr_pool.tile([128, 512], bf16, tag="ptr")
                    for t in range(4):
                        nc.tensor.transpose(
                            ptr[:, t * 128 : (t + 1) * 128],
                            in_tiles[t][:, dc * 128 : (dc + 1) * 128],
                            ident_bf16[:],
                        )
                    dst = inputsT[dc][:, s0 : s0 + 512]
                    if dc % 2 == 0:
                        nc.vector.tensor_copy(dst, ptr[:])
                    else:
                        nc.scalar.copy(dst, ptr[:])

                # qk projection -> qkT (u on partitions), + bias
                pqk = pqk_pool.tile([128, 512], f32, tag="pqk")
                for dc in range(DC):
                    nc.tensor.matmul(
                        pqk[:],
                        wqk_sb[:, dc * 128 : (dc + 1) * 128],
                        inputsT[dc][:, s0 : s0 + 512],
                        start=(dc == 0),
                        stop=(dc == DC - 1),
                    )
                for h in range(HL):
                    nc.vector.tensor_scalar_add(
                        qkT[h][:, s0 : s0 + 512],
                        pqk[h * 64 : (h + 1) * 64, :],
                        bqk_col[h * 64 : (h + 1) * 64, :],
                    )

                # v projection -> vT, + bias, then PE-transpose to [s,u] chunks
                pv = pv_pool.tile([128, 512], f32, tag="pv")
                for dc in range(DC):
                    nc.tensor.matmul(
                        pv[:],
                        wv_sb[:, dc * 128 : (dc + 1) * 128],
                        inputsT[dc][:, s0 : s0 + 512],
                        start=(dc == 0),
                        stop=(dc == DC - 1),
                    )
                vt_sb = vt_pool.tile([128, 512], bf16, tag="vt")
                for h in range(HL):
                    nc.scalar.activation(
                        vt_sb[h * 64 : (h + 1) * 64, :],
                        pv[h * 64 : (h + 1) * 64, :],
                        AF.Identity,
                        bias=bv_col[h * 64 : (h + 1) * 64, :],
                    )
                for h in range(HL):
                    for t in range(4):
                        ci = sb * 4 + t  # global 128-row chunk index in [0,32)
                        pvt = pvt_pool.tile([128, 64], bf16, tag="pvt")
                        nc.tensor.transpose(
                            pvt[:],
                            vt_sb[h * 64 : (h + 1) * 64, t * 128 : (t + 1) * 128],
                            ident_bf16[h * 64 : (h + 1) * 64, h * 64 : (h + 1) * 64],
                        )
                        nc.vector.tensor_copy(
                            vloc[h][:, ci * 64 : (ci + 1) * 64], pvt[:]
                        )

        # ---------------- Phase 2: attention per (b, h) ----------------
        with ExitStack() as p2:
            # PSUM budget (8 banks): scores split lo/hi so score matmuls of
            # q-block i+1 overlap the exp of q-block i: lo [128,1024] x2 bufs
            # (4 banks) + hi [128,1024] x1 (2 banks) + ctx packed into 2 banks
            # (4 chunks of [64,512] on 2 partition-halves x 2 bank-halves).
            scl_pool = p2.enter_context(tc.tile_pool(name="scl", bufs=2, space="PSUM"))
            sch_pool = p2.enter_context(tc.tile_pool(name="sch", bufs=1, space="PSUM"))
            cx_pool = p2.enter_context(tc.tile_pool(name="cx", bufs=1, space="PSUM"))
            e_pool = p2.enter_context(tc.tile_pool(name="e", bufs=3))
            p_pool = p2.enter_context(tc.tile_pool(name="p", bufs=4))
            acc_pool = p2.enter_context(tc.tile_pool(name="acc", bufs=4))
            r_pool = p2.enter_context(tc.tile_pool(name="recip", bufs=2))
            ct_pool = p2.enter_context(tc.tile_pool(name="ctxT", bufs=2))
            co_pool = p2.enter_context(tc.tile_pool(name="cout", bufs=4))

            for b in range(B):
                for h in range(HL):
                    base = b * S
                    recips = r_pool.tile([128, QB], f32, tag="recips")
                    cx = cx_pool.tile([128, S // 2], f32, tag="cx")

                    def cx_sl(c):
                        # ctx chunk c of [64,512] packed: partition-half c%2,
                        # bank-half c//2
                        p0 = (c % 2) * 64
                        f0 = (c // 2) * 512
                        return cx[p0 : p0 + 64, f0 : f0 + 512]

                    def ctx_mms(qb, e_t, b=b, h=h):
                        # context accumulation: E row-tile doubles as E^T k-chunk
                        for c in range(KC):
                            nc.tensor.matmul(
                                cx_sl(c),
                                vloc[h][:, (b * QB + qb) * 64 : (b * QB + qb + 1) * 64],
                                e_t[:, c * 512 : (c + 1) * 512],
                                start=(qb == 0),
                                stop=(qb == QB - 1),
                            )

                    def score_mm(dst, c, qb, lhsT_s, h=h):
                        diag = c == qb // 4
                        nc.tensor.matmul(
                            dst[:, (c % 2) * 512 : (c % 2) * 512 + 512],
                            lhsT_s,
                            qkT[h][:, base + c * 512 : base + (c + 1) * 512],
                            start=True,
                            stop=not diag,
                        )
                        if diag:
                            # diagonal exclusion (pre-exp: row-sums exclude
                            # it): accumulate ident.T @ (-8e4*I) = -8e4*I
                            o = qb * 128 - (c // 2) * 1024
                            nc.tensor.matmul(
                                dst[:, o : o + 128],
                                ident_bf16[:],
                                eyeneg[:],
                                start=False,
                                stop=True,
                            )

                    prev = None  # (qb, e_t) pending context matmuls
                    for qb in range(QB):
                        sc_lo = scl_pool.tile([128, S // 2], f32, tag="scl")
                        sc_hi = sch_pool.tile([128, S // 2], f32, tag="sch")
                        lhsT_s = qkT[h][:, base + qb * 128 : base + (qb + 1) * 128]
                        for c in (0, 1):
                            score_mm(sc_lo, c, qb, lhsT_s)
                        for c in (2, 3):
                            score_mm(sc_hi, c, qb, lhsT_s)
                        e_t = e_pool.tile([128, S], bf16, tag="e")
                        acc_a = acc_pool.tile([128, 1], f32, tag="acca")
                        acc_b = acc_pool.tile([128, 1], f32, tag="accb")
                        nc.scalar.activation(
                            e_t[:, 0 : S // 2], sc_lo[:], AF.Exp, bias=0.0,
                            scale=0.125, accum_out=acc_a[:],
                        )
                        # hi half: skip the ACT accumulator (one READ_ACCUM
                        # per q-block is enough overhead); row-sum the bf16
                        # output on DVE instead
                        nc.scalar.activation(
                            e_t[:, S // 2 : S], sc_hi[:], AF.Exp, bias=0.0,
                            scale=0.125,
                        )
                        # previous q-block's context matmuls: ready PE work
                        # while this exp (and the probs scale) runs
                        if prev is not None:
                            ctx_mms(*prev)
                        prev = (qb, e_t)
                        nc.vector.tensor_reduce(
                            acc_b[:], e_t[:, S // 2 : S], ALU_X, ALU.add
                        )
                        nc.vector.tensor_add(acc_a[:], acc_a[:], acc_b[:])
                        nc.vector.reciprocal(recips[:, qb : qb + 1], acc_a[:])
                        # probs = E * (1/rowsum), streamed to DRAM
                        p_t = p_pool.tile([128, S], probs_dt, tag="p")
                        eng = P_SCALE_ENGINES[qb % len(P_SCALE_ENGINES)]
                        if eng == "s":
                            nc.scalar.activation(
                                p_t[:], e_t[:], AF.Copy,
                                scale=recips[:, qb : qb + 1],
                            )
                        else:
                            # bf16-in/bf16-out tensor_scalar hits the packed
                            # DVE mode (~750ns/tile vs 2.3us for broadcast TT)
                            nc.vector.tensor_scalar_mul(
                                p_t[:], e_t[:], recips[:, qb : qb + 1]
                            )
                        nc.sync.dma_start(
                            out=probs_ext[b, h, qb * 128 : (qb + 1) * 128, :],
                            in_=p_t[:],
                        )
                    ctx_mms(*prev)
                    # finalize context: copy accumulated [u, q] out as-is; the
                    # host transposes + row-normalizes this 2MB side output
                    ctxT_sb = ct_pool.tile([64, S], f32, tag="ctxT")
                    for c in range(KC):
                        nc.scalar.copy(ctxT_sb[:, c * 512 : (c + 1) * 512], cx_sl(c))
                    nc.sync.dma_start(out=ctxt_ext[b, h, :, :], in_=ctxT_sb[:])
                    nc.sync.dma_start(out=recip_ext[b, h, :, :], in_=recips[:])

    nc.compile()
    return nc


def _get_compiled():
    if "nc" not in _COMPILED:
        _COMPILED["nc"] = _build()
    return _COMPILED["nc"]


def _numpy_fallback(inputs, mask, W_qk, b_qk, W_v, b_v):
    H, U = NUM_HEADS, UNITS
    x = inputs.astype(np.float64)
    qk = (x @ W_qk + b_qk).reshape(B, S, H, U).transpose(0, 2, 1, 3)
    v = (x @ W_v + b_v).reshape(B, S, H, U).transpose(0, 2, 1, 3)
    sc = np.einsum("bhqd,bhkd->bhqk", qk, qk) / np.sqrt(np.float64(U))
    m = np.ones((B, S, 1)) * mask.astype(np.float64)[:, None, :]
    m = np.clip(m - np.eye(S), 0.0, 1.0)
    sc = sc + (1.0 - m[:, None, :, :]) * -10000.0
    sc -= sc.max(-1, keepdims=True)
    e = np.exp(sc)
    probs = e / e.sum(-1, keepdims=True)
    ctx = np.einsum("bhqk,bhkd->bhqd", probs, v)
    ctx = ctx.transpose(0, 2, 1, 3).reshape(B, S, H * U)
    return ctx.astype(np.float32), probs.astype(np.float32)


def _make_in_maps(inputs, W_qk, b_qk, W_v, b_v):
    in_maps = []
    for c in range(N_CORES):
        cols = slice(c * HL * UNITS, (c + 1) * HL * UNITS)
        in_maps.append(
            {
                "inputs": np.ascontiguousarray(
                    inputs.reshape(BS, D).astype(np.float32)
                ),
                "w_qk": np.ascontiguousarray(W_qk[:, cols].astype(np.float32)),
                "b_qk": np.ascontiguousarray(
                    b_qk[cols].astype(np.float32).reshape(HL * UNITS, 1)
                ),
                "w_v": np.ascontiguousarray(W_v[:, cols].astype(np.float32)),
                "b_v": np.ascontiguousarray(
                    b_v[cols].astype(np.float32).reshape(HL * UNITS, 1)
                ),
            }
        )
    return in_maps


def _assemble(results):
    probs = np.empty((B, NUM_HEADS, S, S), np.float32)
    ctx = np.empty((B, S, NUM_HEADS * UNITS), np.float32)
    for c in range(N_CORES):
        pr = np.asarray(results[c]["probs"]).astype(np.float32).reshape(B, HL, S, S)
        probs[:, c * HL : (c + 1) * HL] = pr
        ctxT = np.asarray(results[c]["ctxT"]).reshape(B, HL, UNITS, S)
        recips = np.asarray(results[c]["recips"]).reshape(B, HL, 128, S // 128)
        for b in range(B):
            for h in range(HL):
                r = recips[b, h].T.reshape(S)  # r[q], q = qb*128 + p
                col = (c * HL + h) * UNITS
                ctx[b, :, col : col + UNITS] = (ctxT[b, h] * r[None, :]).T
    return ctx, probs


def kernel(inputs, mask, W_qk, b_qk, W_v, b_v):
    inputs = np.asarray(inputs, np.float32)
    mask = np.asarray(mask)
    W_qk = np.asarray(W_qk, np.float32)
    b_qk = np.asarray(b_qk, np.float32)
    W_v = np.asarray(W_v, np.float32)
    b_v = np.asarray(b_v, np.float32)
    if not bool(np.all(mask)):
        # the device kernel hardcodes the all-ones mask fast path
        return _numpy_fallback(inputs, mask, W_qk, b_qk, W_v, b_v)

    from concourse.bass_utils import run_bass_kernel_spmd

    nc = _get_compiled()
    in_maps = _make_in_maps(inputs, W_qk, b_qk, W_v, b_v)
    res = run_bass_kernel_spmd(nc, in_maps, list(range(N_CORES)))
    ctx, probs = _assemble(res.results)
    return ctx, probs


# revision 32
# speedup vs baseline: 1.1620x; 1.0487x over previous
"""Trainium2 Bass kernel for shared-QK attention (Q=K) with probs output.

Reference computation (B=2, S=2048, D=1024, H=16, U=64):
    qk    = inputs @ W_qk + b_qk            -> [B,H,S,U]
    v     = inputs @ W_v  + b_v             -> [B,H,S,U]
    s     = qk @ qk^T / sqrt(U)             -> [B,H,S,S]   (symmetric)
    s    += eye-exclusion mask (-1e4 on diagonal; mask input is all-ones)
    probs = softmax(s, -1)                  -> [B,H,S,S]
    ctx   = probs @ v                       -> [B,S,H*U]
    returns (ctx, probs)

Sharding: 2 heads per core x 8 cores (both batches local), no collectives —
each core emits its probs/context shard as an ExternalOutput and the host
assembles the full tensors.

Per-core kernel design notes:
  - inputs are PE-transposed (f32) into inputsT (bf16), feeding both
    projections with D on partitions.
  - exp(scores) is computed unnormalized (no max subtraction needed: |s|<~4).
    E := exp(s/8) is symmetric, so stored E row-tiles [q,k] double as E^T
    k-chunk tiles [k,q] for the context matmul — no transpose of probs.
  - ACT's fused accum_out on the exp gives the row-sums for free; probs are
    E * (1/rowsum) via per-partition tensor_scalar, context is normalized
    after a PE transpose back to [s,u] layout (again per-partition scalars).
  - diagonal exclusion: add -8e4*I to the score PSUM tile pre-exp, so
    exp((s-8e4)/8) underflows to exactly 0.0 like the reference.
"""

import numpy as np

NUM_HEADS = 16
UNITS = 64
B, S, D = 2, 2048, 1024
N_CORES = 8
HL = NUM_HEADS // N_CORES  # heads per core (2)
BS = B * S  # 4096

# engine for the probs normalization pass per q-block parity:
#   "v" = vector (DVE, tensor_tensor w/ stride-0 broadcast), "s" = scalar (ACT,
#   fused scale on a Copy activation). gpsimd/tensor_scalar-with-AP measured
#   ~11 cyc/elem — do not use.
P_SCALE_ENGINES = ("v",)
# probs DRAM/store dtype: bf16 halves the dominant DMA stream; host converts
# back to f32. Set to "f32" to store probs in full precision.
PROBS_DTYPE = "bf16"

_COMPILED = {}


def _build():
    import concourse.bass as bass  # noqa: F401
    import concourse.mybir as mybir
    from concourse import bacc
    from concourse.tile import TileContext
    from concourse.masks import make_identity
    from contextlib import ExitStack

    dt = mybir.dt
    f32 = dt.float32
    bf16 = dt.bfloat16
    AF = mybir.ActivationFunctionType
    ALU = mybir.AluOpType
    ALU_X = mybir.AxisListType.X

    U = UNITS
    DC = D // 128          # 8 contraction chunks
    NSB = BS // 512        # 8 s-blocks of 512
    QB = S // 128          # 16 q-blocks per (b,h)
    KC = S // 512          # 4 free-dim chunks of 512

    nc = bacc.Bacc("TRN2", target_bir_lowering=False, debug=False)
    inputs_ext = nc.declare_dram_parameter("inputs", [BS, D], f32, isOutput=False)
    wqk_ext = nc.declare_dram_parameter("w_qk", [D, HL * U], f32, isOutput=False)
    bqk_ext = nc.declare_dram_parameter("b_qk", [HL * U, 1], f32, isOutput=False)
    wv_ext = nc.declare_dram_parameter("w_v", [D, HL * U], f32, isOutput=False)
    bv_ext = nc.declare_dram_parameter("b_v", [HL * U, 1], f32, isOutput=False)
    probs_dt = bf16 if PROBS_DTYPE == "bf16" else f32
    probs_ext = nc.declare_dram_parameter(
        "probs", [B, HL, S, S], probs_dt, isOutput=True
    )
    # context is exported transposed+unnormalized ([u, q] straight from PSUM)
    # together with the per-row reciprocals; the host applies the (tiny)
    # transpose + row scaling during unsharding.
    ctxt_ext = nc.declare_dram_parameter(
        "ctxT", [B, HL, U, S], f32, isOutput=True
    )
    recip_ext = nc.declare_dram_parameter(
        "recips", [B, HL, 128, S // 128], f32, isOutput=True
    )

    with TileContext(nc) as tc, ExitStack() as top:
        consts = top.enter_context(tc.tile_pool(name="consts", bufs=1))
        pers = top.enter_context(tc.tile_pool(name="pers", bufs=1))

        ident_f32 = consts.tile([128, 128], f32, tag="idf32")
        make_identity(nc, ident_f32[:])
        ident_bf16 = consts.tile([128, 128], bf16, tag="idbf16")
        make_identity(nc, ident_bf16[:])
        # -8e4 * I as bf16, added into the scores diagonal via a PE matmul
        # (ident.T @ eyeneg) so the exclusion never leaves the TensorEngine
        eyeneg = consts.tile([128, 128], bf16, tag="eyeneg")
        make_identity(nc, eyeneg[:])
        nc.scalar.mul(eyeneg[:], eyeneg[:], -80000.0)

        bqk_col = consts.tile([128, 1], f32, tag="bqk")
        nc.sync.dma_start(out=bqk_col[:], in_=bqk_ext[:, :])
        bv_col = consts.tile([128, 1], f32, tag="bv")
        nc.sync.dma_start(out=bv_col[:], in_=bv_ext[:, :])

        # weights: [D, 128] -> sbuf [128(d within chunk), DC*128(u)] as bf16
        wqk_sb = consts.tile([128, DC * 128], bf16, tag="wqk")
        wv_sb = consts.tile([128, DC * 128], bf16, tag="wv")
        for w_ext, w_sb, tag in ((wqk_ext, wqk_sb, "wqkf"), (wv_ext, wv_sb, "wvf")):
            w_f32 = consts.tile([128, DC * 128], f32, tag=tag)
            for c in range(DC):
                nc.sync.dma_start(
                    out=w_f32[:, c * 128 : (c + 1) * 128],
                    in_=w_ext[c * 128 : (c + 1) * 128, :],
                )
            nc.vector.tensor_copy(w_sb[:], w_f32[:])

        # persistent intermediates
        qkT = [pers.tile([64, BS], bf16, tag=f"qkT{h}", name=f"qkT{h}") for h in range(HL)]
        vloc = [pers.tile([128, (BS // 128) * U], bf16, tag=f"v{h}", name=f"vloc{h}") for h in range(HL)]

        # ---------------- Phase 1: transpose + projections ----------------
        with ExitStack() as p1:
            in_pool = p1.enter_context(tc.tile_pool(name="inp", bufs=8))
            it_pool = p1.enter_context(tc.tile_pool(name="inT", bufs=1))
            vt_pool = p1.enter_context(tc.tile_pool(name="vT", bufs=2))
            ptr_pool = p1.enter_context(tc.tile_pool(name="ptr", bufs=2, space="PSUM"))
            pqk_pool = p1.enter_context(tc.tile_pool(name="pqk", bufs=2, space="PSUM"))
            pv_pool = p1.enter_context(tc.tile_pool(name="pv", bufs=2, space="PSUM"))
            pvt_pool = p1.enter_context(tc.tile_pool(name="pvt", bufs=2, space="PSUM"))

            inputsT = [
                it_pool.tile([128, BS], bf16, tag=f"inT{dc}", name=f"inT{dc}")
                for dc in range(DC)
            ]

            for sb in range(NSB):
                s0 = sb * 512
                in_tiles = []
                for t in range(4):
                    it = in_pool.tile([128, D], f32, tag="in")
                    nc.sync.dma_start(
                        out=it[:], in_=inputs_ext[s0 + t * 128 : s0 + (t + 1) * 128, :]
                    )
                    # cast to bf16 up front: PE transposes run 2x faster on
                    # bf16 and the projections consume bf16 anyway
                    ib = in_pool.tile([128, D], bf16, tag="inb")
                    if t % 2 == 0:
                        nc.vector.tensor_copy(ib[:], it[:])
                    else:
                        nc.scalar.copy(ib[:], it[:])
                    in_tiles.append(ib)

                # transpose this s-block: inputsT[dc][:, s0:s0+512]
                for dc in range(DC):
                    ptr = ptr_pool.tile([128, 512], bf16, tag="ptr")
                    for t in range(4):
                        nc.tensor.transpose(
                            ptr[:, t * 128 : (t + 1) * 128],
                            in_tiles[t][:, dc * 128 : (dc + 1) * 128],
                            ident_bf16[:],
                        )
                    dst = inputsT[dc][:, s0 : s0 + 512]
                    if dc % 2 == 0:
                        nc.vector.tensor_copy(dst, ptr[:])
                    else:
                        nc.scalar.copy(dst, ptr[:])

                # qk projection -> qkT (u on partitions), + bias
                pqk = pqk_pool.tile([128, 512], f32, tag="pqk")
                for dc in range(DC):
                    nc.tensor.matmul(
                        pqk[:],
                        wqk_sb[:, dc * 128 : (dc + 1) * 128],
                        inputsT[dc][:, s0 : s0 + 512],
                        start=(dc == 0),
                        stop=(dc == DC - 1),
                    )
                for h in range(HL):
                    nc.vector.tensor_scalar_add(
                        qkT[h][:, s0 : s0 + 512],
                        pqk[h * 64 : (h + 1) * 64, :],
                        bqk_col[h * 64 : (h + 1) * 64, :],
                    )

                # v projection -> vT, + bias, then PE-transpose to [s,u] chunks
                pv = pv_pool.tile([128, 512], f32, tag="pv")
                for dc in range(DC):
                    nc.tensor.matmul(
                        pv[:],
                        wv_sb[:, dc * 128 : (dc + 1) * 128],
                        inputsT[dc][:, s0 : s0 + 512],
                        start=(dc == 0),
                        stop=(dc == DC - 1),
                    )
                vt_sb = vt_pool.tile([128, 512], bf16, tag="vt")
                for h in range(HL):
                    nc.scalar.activation(
                        vt_sb[h * 64 : (h + 1) * 64, :],
                        pv[h * 64 : (h + 1) * 64, :],
                        AF.Identity,
                        bias=bv_col[h * 64 : (h + 1) * 64, :],
                    )
                for h in range(HL):
                    for t in range(4):
                        ci = sb * 4 + t  # global 128-row chunk index in [0,32)
                        pvt = pvt_pool.tile([128, 64], bf16, tag="pvt")
                        nc.tensor.transpose(
                            pvt[:],
                            vt_sb[h * 64 : (h + 1) * 64, t * 128 : (t + 1) * 128],
                            ident_bf16[h * 64 : (h + 1) * 64, h * 64 : (h + 1) * 64],
                        )
                        nc.vector.tensor_copy(
                            vloc[h][:, ci * 64 : (ci + 1) * 64], pvt[:]
                        )

        # ---------------- Phase 2: attention per (b, h) ----------------
        with ExitStack() as p2:
            # PSUM budget (8 banks): scores split lo/hi so score matmuls of
            # q-block i+1 overlap the exp of q-block i: lo [128,1024] x2 bufs
            # (4 banks) + hi [128,1024] x1 (2 banks) + ctx packed into 2 banks
            # (4 chunks of [64,512] on 2 partition-halves x 2 bank-halves).
            scl_pool = p2.enter_context(tc.tile_pool(name="scl", bufs=2, space="PSUM"))
            sch_pool = p2.enter_context(tc.tile_pool(name="sch", bufs=1, space="PSUM"))
            cx_pool = p2.enter_context(tc.tile_pool(name="cx", bufs=1, space="PSUM"))
            e_pool = p2.enter_context(tc.tile_pool(name="e", bufs=4))
            p_pool = p2.enter_context(tc.tile_pool(name="p", bufs=6))
            acc_pool = p2.enter_context(tc.tile_pool(name="acc", bufs=4))
            r_pool = p2.enter_context(tc.tile_pool(name="recip", bufs=2))
            ct_pool = p2.enter_context(tc.tile_pool(name="ctxT", bufs=2))

            for b in range(B):
                for h in range(HL):
                    base = b * S
                    recips = r_pool.tile([128, QB], f32, tag="recips")
                    cx = cx_pool.tile([128, S // 2], f32, tag="cx")

                    def cx_sl(c):
                        # ctx chunk c of [64,512] packed: partition-half c%2,
                        # bank-half c//2
                        p0 = (c % 2) * 64
                        f0 = (c // 2) * 512
                        return cx[p0 : p0 + 64, f0 : f0 + 512]

                    def ctx_mms(qb, e_t, b=b, h=h):
                        # context accumulation: E row-tile doubles as E^T k-chunk
                        for c in range(KC):
                            nc.tensor.matmul(
                                cx_sl(c),
                                vloc[h][:, (b * QB + qb) * 64 : (b * QB + qb + 1) * 64],
                                e_t[:, c * 512 : (c + 1) * 512],
                                start=(qb == 0),
                                stop=(qb == QB - 1),
                            )

                    def score_mm(dst, c, qb, lhsT_s, h=h):
                        diag = c == qb // 4
                        nc.tensor.matmul(
                            dst[:, (c % 2) * 512 : (c % 2) * 512 + 512],
                            lhsT_s,
                            qkT[h][:, base + c * 512 : base + (c + 1) * 512],
                            start=True,
                            stop=not diag,
                        )
                        if diag:
                            # diagonal exclusion (pre-exp: row-sums exclude
                            # it): accumulate ident.T @ (-8e4*I) = -8e4*I
                            o = qb * 128 - (c // 2) * 1024
                            nc.tensor.matmul(
                                dst[:, o : o + 128],
                                ident_bf16[:],
                                eyeneg[:],
                                start=False,
                                stop=True,
                            )

                    prev = None  # (qb, e_t) pending context matmuls
                    for qb in range(QB):
                        sc_lo = scl_pool.tile([128, S // 2], f32, tag="scl")
                        sc_hi = sch_pool.tile([128, S // 2], f32, tag="sch")
                        lhsT_s = qkT[h][:, base + qb * 128 : base + (qb + 1) * 128]
                        for c in (0, 1):
                            score_mm(sc_lo, c, qb, lhsT_s)
                        for c in (2, 3):
                            score_mm(sc_hi, c, qb, lhsT_s)
                        e_t = e_pool.tile([128, S], bf16, tag="e")
                        acc_a = acc_pool.tile([128, 1], f32, tag="acca")
                        acc_b = acc_pool.tile([128, 1], f32, tag="accb")
                        nc.scalar.activation(
                            e_t[:, 0 : S // 2], sc_lo[:], AF.Exp, bias=0.0,
                            scale=0.125, accum_out=acc_a[:],
                        )
                        # hi half: skip the ACT accumulator (one READ_ACCUM
                        # per q-block is enough overhead); row-sum the bf16
                        # output on DVE instead
                        nc.scalar.activation(
                            e_t[:, S // 2 : S], sc_hi[:], AF.Exp, bias=0.0,
                            scale=0.125,
                        )
                        # previous q-block's context matmuls: ready PE work
                        # while this exp (and the probs scale) runs
                        if prev is not None:
                            ctx_mms(*prev)
                        prev = (qb, e_t)
                        nc.vector.tensor_reduce(
                            acc_b[:], e_t[:, S // 2 : S], ALU_X, ALU.add
                        )
                        nc.vector.tensor_add(acc_a[:], acc_a[:], acc_b[:])
                        nc.vector.reciprocal(recips[:, qb : qb + 1], acc_a[:])
                        # probs = E * (1/rowsum), streamed to DRAM
                        p_t = p_pool.tile([128, S], probs_dt, tag="p")
                        eng = P_SCALE_ENGINES[qb % len(P_SCALE_ENGINES)]
                        if eng == "s":
                            nc.scalar.activation(
                                p_t[:], e_t[:], AF.Copy,
                                scale=recips[:, qb : qb + 1],
                            )
                        else:
                            # bf16-in/bf16-out tensor_scalar hits the packed
                            # DVE mode (~750ns/tile vs 2.3us for broadcast TT)
                            nc.vector.tensor_scalar_mul(
                                p_t[:], e_t[:], recips[:, qb : qb + 1]
                            )
                        nc.sync.dma_start(
                            out=probs_ext[b, h, qb * 128 : (qb + 1) * 128, :],
                            in_=p_t[:],
                        )
                    ctx_mms(*prev)
                    # finalize context: copy accumulated [u, q] out as-is; the
                    # host transposes + row-normalizes this 2MB side output
                    ctxT_sb = ct_pool.tile([64, S], f32, tag="ctxT")
                    for c in range(KC):
                        nc.scalar.copy(ctxT_sb[:, c * 512 : (c + 1) * 512], cx_sl(c))
                    nc.sync.dma_start(out=ctxt_ext[b, h, :, :], in_=ctxT_sb[:])
                    nc.sync.dma_start(out=recip_ext[b, h, :, :], in_=recips[:])

    nc.compile()
    return nc


def _get_compiled():
    if "nc" not in _COMPILED:
        _COMPILED["nc"] = _build()
    return _COMPILED["nc"]


def _numpy_fallback(inputs, mask, W_qk, b_qk, W_v, b_v):
    H, U = NUM_HEADS, UNITS
    x = inputs.astype(np.float64)
    qk = (x @ W_qk + b_qk).reshape(B, S, H, U).transpose(0, 2, 1, 3)
    v = (x @ W_v + b_v).reshape(B, S, H, U).transpose(0, 2, 1, 3)
    sc = np.einsum("bhqd,bhkd->bhqk", qk, qk) / np.sqrt(np.float64(U))
    m = np.ones((B, S, 1)) * mask.astype(np.float64)[:, None, :]
    m = np.clip(m - np.eye(S), 0.0, 1.0)
    sc = sc + (1.0 - m[:, None, :, :]) * -10000.0
    sc -= sc.max(-1, keepdims=True)
    e = np.exp(sc)
    probs = e / e.sum(-1, keepdims=True)
    ctx = np.einsum("bhqk,bhkd->bhqd", probs, v)
    ctx = ctx.transpose(0, 2, 1, 3).reshape(B, S, H * U)
    return ctx.astype(np.float32), probs.astype(np.float32)


def _make_in_maps(inputs, W_qk, b_qk, W_v, b_v):
    in_maps = []
    for c in range(N_CORES):
        cols = slice(c * HL * UNITS, (c + 1) * HL * UNITS)
        in_maps.append(
            {
                "inputs": np.ascontiguousarray(
                    inputs.reshape(BS, D).astype(np.float32)
                ),
                "w_qk": np.ascontiguousarray(W_qk[:, cols].astype(np.float32)),
                "b_qk": np.ascontiguousarray(
                    b_qk[cols].astype(np.float32).reshape(HL * UNITS, 1)
                ),
                "w_v": np.ascontiguousarray(W_v[:, cols].astype(np.float32)),
                "b_v": np.ascontiguousarray(
                    b_v[cols].astype(np.float32).reshape(HL * UNITS, 1)
                ),
            }
        )
    return in_maps


def _assemble(results):
    probs = np.empty((B, NUM_HEADS, S, S), np.float32)
    ctx = np.empty((B, S, NUM_HEADS * UNITS), np.float32)
    for c in range(N_CORES):
        pr = np.asarray(results[c]["probs"]).astype(np.float32).reshape(B, HL, S, S)
        probs[:, c * HL : (c + 1) * HL] = pr
        ctxT = np.asarray(results[c]["ctxT"]).reshape(B, HL, UNITS, S)
        recips = np.asarray(results[c]["recips"]).reshape(B, HL, 128, S // 128)
        for b in range(B):
            for h in range(HL):
                r = recips[b, h].T.reshape(S)  # r[q], q = qb*128 + p
                col = (c * HL + h) * UNITS
                ctx[b, :, col : col + UNITS] = (ctxT[b, h] * r[None, :]).T
    return ctx, probs


def kernel(inputs, mask, W_qk, b_qk, W_v, b_v):
    inputs = np.asarray(inputs, np.float32)
    mask = np.asarray(mask)
    W_qk = np.asarray(W_qk, np.float32)
    b_qk = np.asarray(b_qk, np.float32)
    W_v = np.asarray(W_v, np.float32)
    b_v = np.asarray(b_v, np.float32)
    if not bool(np.all(mask)):
        # the device kernel hardcodes the all-ones mask fast path
        return _numpy_fallback(inputs, mask, W_qk, b_qk, W_v, b_v)

    from concourse.bass_utils import run_bass_kernel_spmd

    nc = _get_compiled()
    in_maps = _make_in_maps(inputs, W_qk, b_qk, W_v, b_v)
    res = run_bass_kernel_spmd(nc, in_maps, list(range(N_CORES)))
    ctx, probs = _assemble(res.results)
    return ctx, probs
